# revision 2
# baseline (speedup 1.0000x reference)
"""Multi-head attention (B=2, S=2048, D=1024, H=16) on 8 Trainium2 NeuronCores.

Tensor-parallel over heads (2 per core); host sums the 8 fp16 partials and
adds biases. All-fp16 matmuls; kernel is PE-bound (~167us of matmul), so the
schedule keeps the Tensor engine continuously busy (also holding DVFS at
full clock):

  - Host pre-packs x / weights in SBUF-layout so every DMA is contiguous;
    x tile loads are split 4-ways across DMA queues.
  - Q/K projection feat-major; Q eviction folds bias + 1/8 scale; K bias
    dropped (softmax is invariant to per-query logit shifts); V bias folded
    on the host (attention weights sum to 1 -> constant w_fc @ b_v).
  - V projection token-major, 4 token-tiles per psum, evicted into per-tb
    key-major tiles vkb = [V_h0 | ones | V_h1] (overlapping 128-col slices
    give each head's [V|ones] AV operand; ones produce the softmax
    denominators in the spare output partitions).
  - Attention per (batch, query-block): 16 key-tile steps of scoresT
    [keys, 2x512] -> one wide exp -> fp16 et -> AV trailing by 2 tiles.
  - Normalization: cross-partition moves via DVE stream_shuffle (no DMA),
    reciprocal + elementwise multiply into valuesT fp16.
  - FC partial -> fp16 out, output DMAs split in half across queues.
  - Projection of the next batch and FC of finished blocks are emitted as
    cost-estimated work units, drained between attention steps at a rate
    that keeps the PE oversubscribed relative to the Act engine (exp).
"""
import numpy as np
from contextlib import ExitStack

import concourse.bass as bass
import concourse.tile as tile
from concourse import bacc, mybir
from concourse.bass_utils import run_bass_kernel_spmd

B, S, D, H, HD = 2, 2048, 1024, 16, 64
T = B * S
NC = 8
HPC = H // NC
F = HPC * HD             # 128
KT = 128
QB = 512
NKT = S // KT            # 16
TPB = T // QB            # 8
NDT = D // 128           # 8 contraction tiles
f32 = mybir.dt.float32
fp16 = mybir.dt.float16
AF = mybir.ActivationFunctionType
OP = mybir.AluOpType

AV_LAG = 4
USE_SHUFFLE = True
IDENT32 = list(range(32))

_NC_CACHE = None


def _build():
    nc = bacc.Bacc("TRN2", target_bir_lowering=False, debug=False, num_devices=NC)

    # host-prepped layouts: contiguous per-partition DMAs
    X = nc.dram_tensor("x", [TPB, 128, NDT, QB], fp16, kind="ExternalInput").ap()
    WQ = nc.dram_tensor("wq", [128, NDT, F], fp16, kind="ExternalInput").ap()
    WK = nc.dram_tensor("wk", [128, NDT, F], fp16, kind="ExternalInput").ap()
    WV = nc.dram_tensor("wv", [128, NDT, F], fp16, kind="ExternalInput").ap()
    BQ = nc.dram_tensor("bq", [F, 1], f32, kind="ExternalInput").ap()
    WFC = nc.dram_tensor("wfc", [F, D], fp16, kind="ExternalInput").ap()
    OUT = nc.dram_tensor("out", [T, D], fp16, kind="ExternalOutput").ap()

    with tile.TileContext(nc) as tc, ExitStack() as ctx:
        const = ctx.enter_context(tc.tile_pool(name="const", bufs=1))
        big = ctx.enter_context(tc.tile_pool(name="big", bufs=1))
        et_pool = ctx.enter_context(tc.tile_pool(name="etp", bufs=6))
        r_pool = ctx.enter_context(tc.tile_pool(name="recip", bufs=2))
        fout_pool = ctx.enter_context(tc.tile_pool(name="fout", bufs=6))
        ps_pool = ctx.enter_context(tc.tile_pool(name="ps_pool", bufs=1,
                                                 space="PSUM"))

        # weights + x loads split finely across DMA queues (one queue moves
        # ~22.5 GB/s, so a monolithic 256KB+ DMA gates the pipeline start).
        wk_sb = const.tile([128, NDT, F], fp16)
        wq_sb = const.tile([128, NDT, F], fp16)
        wv_sb = const.tile([128, NDT, F], fp16)
        issuers = [nc.sync, nc.scalar]
        syncs = [nc.sync, nc.sync]
        xts = [big.tile([128, NDT, QB], fp16, name=f"xt{tb}")
               for tb in range(TPB)]
        for j in range(4):  # wk first (needed by the very first matmul)
            issuers[j % 2].dma_start(out=wk_sb[:, 2 * j:2 * j + 2, :],
                                     in_=WK[:, 2 * j:2 * j + 2, :])
        for j in range(NDT):  # xt0/xt1 8-way, issue spread over 2 engines
            issuers[j % 2].dma_start(out=xts[0][:, j:j + 1, :],
                                     in_=X[0][:, j:j + 1, :])
        for j in range(4):
            nc.scalar.dma_start(out=wq_sb[:, 2 * j:2 * j + 2, :],
                                in_=WQ[:, 2 * j:2 * j + 2, :])
            nc.sync.dma_start(out=wv_sb[:, 2 * j:2 * j + 2, :],
                              in_=WV[:, 2 * j:2 * j + 2, :])
        for j in range(NDT):
            issuers[j % 2].dma_start(out=xts[1][:, j:j + 1, :],
                                     in_=X[1][:, j:j + 1, :])
        bq_sb = const.tile([F, 1], f32)
        nc.scalar.dma_start(out=bq_sb, in_=BQ)
        for j in range(NDT):
            nc.sync.dma_start(out=xts[2][:, j:j + 1, :],
                              in_=X[2][:, j:j + 1, :])
        for tb in range(3, TPB):
            for j in range(NDT):
                nc.sync.dma_start(out=xts[tb][:, j:j + 1, :],
                                  in_=X[tb][:, j:j + 1, :])
        wfc_sb = const.tile([F, D], fp16)
        for j in range(4):
            nc.sync.dma_start(out=wfc_sb[:, j * 256:(j + 1) * 256],
                              in_=WFC[:, j * 256:(j + 1) * 256])

        qTs = [big.tile([128, QB], fp16, name=f"qT{i}") for i in range(TPB)]
        kTs = [big.tile([128, QB], fp16, name=f"kT{i}") for i in range(TPB)]
        # per token-block key-major V: [128 keys, 4 key-tiles, 192]
        # cols: [V_h0(0:64) | ones(64:128) | V_h1(128:192)]
        vkbs = [big.tile([128, 4, 192], fp16, name=f"vkb{i}")
                for i in range(TPB)]
        for vkb in vkbs:
            nc.gpsimd.memset(vkb[:, :, 64:128], 1.0)
        valuesTs = [big.tile([128, QB], fp16, name=f"valT{i}")
                    for i in range(TPB)]

        # ---------------- emission helpers ----------------
        def emit_qk_proj(tb, which):
            w_sb = wq_sb if which == "q" else wk_sb
            ps = ps_pool.tile([128, QB], f32, tag="mm", bufs=2,
                              name=f"{which}ps{tb}")
            for dt_ in range(NDT):
                nc.tensor.matmul(ps, w_sb[:, dt_, :], xts[tb][:, dt_, :],
                                 start=(dt_ == 0), stop=(dt_ == NDT - 1))
            if which == "q":
                nc.vector.tensor_scalar(qTs[tb], ps, bq_sb, 0.125,
                                        op0=OP.add, op1=OP.mult)
            else:
                nc.vector.tensor_copy(kTs[tb], ps)

        def emit_v_proj_half(tb, h, st):
            """V projection half (2 token-tiles); h=1 also evicts."""
            if h == 0:
                st["ps"] = ps_pool.tile([128, QB], f32, tag="mm", bufs=2,
                                        name=f"vps{tb}")
            ps = st["ps"]
            for sub in (2 * h, 2 * h + 1):
                for dt_ in range(NDT):
                    nc.tensor.matmul(
                        ps[:, sub * 128:(sub + 1) * 128],
                        xts[tb][:, dt_, sub * 128:(sub + 1) * 128],
                        wv_sb[:, dt_, :],
                        start=(dt_ == 0), stop=(dt_ == NDT - 1))
            if h == 1:
                vkb = vkbs[tb]
                psv = ps.rearrange("p (a c) -> p a c", c=128)
                nc.vector.tensor_copy(vkb[:, :, 0:64], psv[:, :, 0:64])
                nc.vector.tensor_copy(vkb[:, :, 128:192], psv[:, :, 64:128])

        def emit_v_proj(tb):
            st = {}
            emit_v_proj_half(tb, 0, st)
            emit_v_proj_half(tb, 1, st)

        def emit_norm(blk, pav, c0, c1):
            vt = valuesTs[blk]
            n = c1 - c0
            den0 = r_pool.tile([64, n], f32, tag="den0", name=f"den0_{blk}_{c0}")
            nc.vector.stream_shuffle(den0, pav[0][64:128, c0:c1], IDENT32)
            rec0 = r_pool.tile([64, n], f32, tag="rec0", name=f"rec0_{blk}_{c0}")
            nc.vector.reciprocal_approx_fast(out=rec0, in_=den0)
            nc.vector.tensor_mul(vt[0:64, c0:c1], pav[0][0:64, c0:c1], rec0)
            rec1 = r_pool.tile([64, n], f32, tag="rec1", name=f"rec1_{blk}_{c0}")
            nc.vector.reciprocal_approx_fast(out=rec1, in_=pav[1][0:64, c0:c1])
            rec1b = r_pool.tile([128, n], f32, tag="rec1b",
                                name=f"rec1b_{blk}_{c0}")
            nc.vector.stream_shuffle(rec1b[64:128, :], rec1, IDENT32)
            nc.vector.tensor_mul(vt[64:128, c0:c1], pav[1][64:128, c0:c1],
                                 rec1b[64:128, :])

        fc_in_tail = [False]

        def emit_fc(b, tb2, eb):
            tt = b * S + tb2 * 128
            fp = ps_pool.tile([128, QB], f32, tag="mm", bufs=2,
                              name=f"fp{b}_{tb2}_{eb}")
            nc.tensor.matmul(
                fp,
                valuesTs[b * 4 + tb2 // 4][:, (tb2 % 4) * 128:(tb2 % 4 + 1) * 128],
                wfc_sb[:, eb * QB:(eb + 1) * QB],
                start=True, stop=True)
            fo = fout_pool.tile([128, QB], fp16, tag="fout",
                                name=f"fo{b}_{tb2}_{eb}")
            if fc_in_tail[0] and eb % 2 == 1:
                nc.scalar.copy(fo, fp)
            else:
                nc.vector.tensor_copy(fo, fp)
            # partition-half split keeps 1KB descriptors with 2x parallelism
            nc.sync.dma_start(out=OUT[tt:tt + 64, eb * QB:(eb + 1) * QB],
                              in_=fo[0:64, :])
            iss2 = nc.scalar if fc_in_tail[0] else nc.sync
            iss2.dma_start(out=OUT[tt + 64:tt + 128, eb * QB:(eb + 1) * QB],
                           in_=fo[64:128, :])

        # ---------------- static per-block work plan ----------------
        # plan[blk] = list of (cost_ns, fn); drained evenly over the block's
        # 16 kt steps.
        plan = [[] for _ in range(9)]  # plan[8] = tail

        class Pacer:
            def __init__(self):
                self.units = []
                self.done = 0.0
                self.total = 0.0
                self.kt = 0

            def block_start(self, blk):
                self.units = list(plan[blk])
                self.total = sum(u[0] for u in self.units) or 1.0
                self.done = 0.0
                self.kt = 0

            def step(self):
                self.kt += 1
                target = self.total * self.kt / (2 * NKT)
                while self.units and self.done < target:
                    c, fn = self.units.pop(0)
                    fn()
                    self.done += c

            def flush(self):
                for _, fn in self.units:
                    fn()
                self.units = []

        pacer = Pacer()

        def emit_attn_block(blk, prev_epilogue=None):
            b, qb = blk // 4, blk % 4
            qTq = qTs[b * 4 + qb]
            pav = [ps_pool.tile([128, QB], f32, tag=f"pav{h}",
                                name=f"pav{h}_{blk}") for h in range(HPC)]
            pending = []

            def emit_av(kt, et):
                vkb = vkbs[b * 4 + kt // 4]
                for h in range(HPC):
                    nc.tensor.matmul(
                        pav[h], vkb[:, kt % 4, h * 64:h * 64 + 128],
                        et[:, h * QB:(h + 1) * QB],
                        start=(kt == 0), stop=(kt == NKT - 1))

            for kt in range(NKT):
                # scores first: the exp chain is the pace-setter, so its
                # input must be produced as early as possible in the step.
                kTk = kTs[b * 4 + kt // 4]
                k0 = (kt % 4) * KT
                sc = ps_pool.tile([128, 2 * QB], f32, tag="sc", bufs=2,
                                  name=f"sc{blk}_{kt}")
                for h in range(HPC):
                    hp = h * HD
                    nc.tensor.matmul(
                        sc[:, h * QB:(h + 1) * QB],
                        kTk[hp:hp + HD, k0:k0 + KT],
                        qTq[hp:hp + HD, :],
                        start=True, stop=True,
                        tile_position=(hp, 0))
                et = et_pool.tile([128, 2 * QB], fp16, tag="et",
                                  name=f"et{blk}_{kt}")
                nc.scalar.activation(et, sc, AF.Exp)
                pending.append((kt, et))
                if kt == 0 and prev_epilogue is not None:
                    # previous block's last AV + norm ride here, after this
                    # block's first scores are already feeding the Act engine
                    prev_epilogue()
                if pending and kt >= AV_LAG:
                    emit_av(*pending.pop(0))
                if kt >= NKT - 3 and pending:
                    emit_av(*pending.pop(0))
                pacer.step()
                pacer.step()

            def epilogue():
                for item in pending:
                    emit_av(*item)
                emit_norm(blk, pav, 0, QB)
            if blk < 7:
                return epilogue

            for item in pending:
                emit_av(*item)
            if True:
                # final block: normalize in halves so the tail FC overlaps
                fc_in_tail[0] = True
                emit_norm(blk, pav, 0, QB // 2)
                for t in (12, 13):
                    for e in range(D // QB):
                        emit_fc(1, t, e)
                emit_norm(blk, pav, QB // 2, QB)

        # ---------------- master schedule ----------------
        C_PROJ, C_V, C_FC = 1900, 2600, 300

        def qk(tb, w):
            return (C_PROJ, lambda: emit_qk_proj(tb, w))

        def vp(tb):
            st = {}

            def half(h):
                def go():
                    emit_v_proj_half(tb, h, st)
                return go
            return [(C_V // 2, half(0)), (C_V // 2, half(1))]

        def fcs(blk):
            b, qb = blk // 4, blk % 4
            return [(C_FC, lambda b=b, t=t, e=e: emit_fc(b, t, e))
                    for t in range(qb * 4, qb * 4 + 4) for e in range(D // QB)]

        # pre: the minimum to start attention (first scores need K0+Q0,
        # first AVs need V0); everything else drains as paced units
        _pre = nc.named_scope("pre")
        _pre.__enter__()
        emit_qk_proj(0, "k")
        emit_qk_proj(0, "q")
        emit_v_proj(0)
        _pre.__exit__(None, None, None)

        plan[0] = ([qk(1, "k")] + vp(1) + [qk(2, "k")] + vp(2)
                   + [qk(3, "k")] + vp(3) + [qk(1, "q")])
        plan[1] = [qk(2, "q"), qk(4, "k")] + vp(4)
        plan[2] = [qk(3, "q"), qk(5, "k")] + vp(5) + [qk(6, "k")]
        plan[3] = [qk(4, "q")] + vp(6) + [qk(7, "k")] + vp(7)
        allfc = fcs(0) + fcs(1) + fcs(2) + fcs(3) + fcs(4) + fcs(5) + fcs(6)
        plan[4] = [qk(5, "q")] + allfc[0:16]
        plan[5] = [qk(6, "q")] + allfc[16:30]
        plan[6] = [qk(7, "q")] + allfc[30:44]
        plan[7] = allfc[44:56]
        plan[8] = [(C_FC, lambda t=t, e=e: emit_fc(1, t, e))
                   for t in (14, 15) for e in range(D // QB)]

        epi = None
        for blk in range(8):
            scope = nc.named_scope(f"attn{blk // 4}{blk % 4}")
            scope.__enter__()
            pacer.block_start(blk)
            epi = emit_attn_block(blk, prev_epilogue=epi)
            pacer.flush()
            scope.__exit__(None, None, None)
        _tail = nc.named_scope("tail")
        _tail.__enter__()
        fc_in_tail[0] = True
        for _, fn in plan[8]:
            fn()
        _tail.__exit__(None, None, None)

    nc.compile()
    return nc


def _get_nc():
    global _NC_CACHE
    if _NC_CACHE is None:
        _NC_CACHE = _build()
    return _NC_CACHE


def _prep_in_maps(x, w_qkv, b_qkv, w_fc):
    # x: [B,S,D] -> [TPB, 128, NDT, QB] : x[tb,p,dt,t] = xT[dt*128+p, tb*QB+t]
    xT = x.reshape(T, D).T.astype(np.float16)              # [D, T]
    xh = np.ascontiguousarray(
        xT.reshape(NDT, 128, TPB, QB).transpose(2, 1, 0, 3))

    def wlay(w):  # [D, F] -> [128, NDT, F]
        return np.ascontiguousarray(
            w.reshape(NDT, 128, F).transpose(1, 0, 2)).astype(np.float16)

    in_maps = []
    for c in range(NC):
        heads = [HPC * c + i for i in range(HPC)]
        rows = {
            "q": np.concatenate([np.arange(h * 3 * HD, h * 3 * HD + HD)
                                 for h in heads]),
            "k": np.concatenate([np.arange(h * 3 * HD + HD, h * 3 * HD + 2 * HD)
                                 for h in heads]),
            "v": np.concatenate([np.arange(h * 3 * HD + 2 * HD, h * 3 * HD + 3 * HD)
                                 for h in heads]),
        }
        m = {
            "x": xh,
            "wq": wlay(w_qkv[rows["q"]].T.astype(np.float16)),
            "wk": wlay(w_qkv[rows["k"]].T.astype(np.float16)),
            "wv": wlay(w_qkv[rows["v"]].T.astype(np.float16)),
            "bq": np.ascontiguousarray(b_qkv[rows["q"]][:, None]).astype(np.float32),
            "wfc": np.ascontiguousarray(
                w_fc[:, c * F:(c + 1) * F].T).astype(np.float16),
        }
        in_maps.append(m)
    return in_maps


def run_kernel(inputs, trace=False, trace_cores=None):
    x = np.asarray(inputs["x"], np.float32)
    w_qkv = np.asarray(inputs["w_qkv"], np.float32)
    b_qkv = np.asarray(inputs["b_qkv"], np.float32)
    w_fc = np.asarray(inputs["w_fc"], np.float32)
    b_fc = np.asarray(inputs["b_fc"], np.float32)

    nc = _get_nc()
    in_maps = _prep_in_maps(x, w_qkv, b_qkv, w_fc)
    res = run_bass_kernel_spmd(
        nc, in_maps, core_ids=list(range(NC)), trace=trace,
        trace_cores=trace_cores,
    )
    out = np.zeros((T, D), np.float32)
    for r in res.results:
        out += np.asarray(r["out"], np.float32)
    b_v_full = np.concatenate(
        [b_qkv[h * 3 * HD + 2 * HD:h * 3 * HD + 3 * HD] for h in range(H)])
    out += b_fc[None, :] + (w_fc @ b_v_full)[None, :]
    return out.reshape(B, S, D), res


def kernel(**inputs):
    out, _ = run_kernel(inputs, trace=False)
    return out


# revision 3
# speedup vs baseline: 1.0337x; 1.0337x over previous
"""Multi-head attention (B=2, S=2048, D=1024, H=16) on 8 Trainium2 NeuronCores.

Tensor-parallel over heads (2 per core); host sums the 8 fp16 partials and
adds biases. All-fp16 matmuls; kernel is PE-bound (~167us of matmul), so the
schedule keeps the Tensor engine continuously busy (also holding DVFS at
full clock):

  - Host pre-packs x / weights in SBUF-layout so every DMA is contiguous;
    x tile loads are split 4-ways across DMA queues.
  - Q/K projection feat-major; Q eviction folds bias + 1/8 scale; K bias
    dropped (softmax is invariant to per-query logit shifts); V bias folded
    on the host (attention weights sum to 1 -> constant w_fc @ b_v).
  - V projection token-major, 4 token-tiles per psum, evicted into per-tb
    key-major tiles vkb = [V_h0 | ones | V_h1] (overlapping 128-col slices
    give each head's [V|ones] AV operand; ones produce the softmax
    denominators in the spare output partitions).
  - Attention per (batch, query-block): 16 key-tile steps of scoresT
    [keys, 2x512] -> one wide exp -> fp16 et -> AV trailing by 2 tiles.
  - Normalization: cross-partition moves via DVE stream_shuffle (no DMA),
    reciprocal + elementwise multiply into valuesT fp16.
  - FC partial -> fp16 out, output DMAs split in half across queues.
  - Projection of the next batch and FC of finished blocks are emitted as
    cost-estimated work units, drained between attention steps at a rate
    that keeps the PE oversubscribed relative to the Act engine (exp).
"""
import numpy as np
from contextlib import ExitStack

import concourse.bass as bass
import concourse.tile as tile
from concourse import bacc, mybir
from concourse.bass_utils import run_bass_kernel_spmd

B, S, D, H, HD = 2, 2048, 1024, 16, 64
T = B * S
NC = 8
HPC = H // NC
F = HPC * HD             # 128
KT = 128
QB = 512
NKT = S // KT            # 16
TPB = T // QB            # 8
NDT = D // 128           # 8 contraction tiles
f32 = mybir.dt.float32
fp16 = mybir.dt.float16
AF = mybir.ActivationFunctionType
OP = mybir.AluOpType

AV_LAG = 4
USE_SHUFFLE = True
IDENT32 = list(range(32))

_NC_CACHE = None


def _build():
    nc = bacc.Bacc("TRN2", target_bir_lowering=False, debug=False, num_devices=NC)

    # host-prepped layouts: contiguous per-partition DMAs
    X = nc.dram_tensor("x", [TPB, 128, NDT, QB], fp16, kind="ExternalInput").ap()
    WQ = nc.dram_tensor("wq", [128, NDT, F], fp16, kind="ExternalInput").ap()
    WK = nc.dram_tensor("wk", [128, NDT, F], fp16, kind="ExternalInput").ap()
    WV = nc.dram_tensor("wv", [128, NDT, F], fp16, kind="ExternalInput").ap()
    BQ = nc.dram_tensor("bq", [F, 1], f32, kind="ExternalInput").ap()
    WFC = nc.dram_tensor("wfc", [F, D], fp16, kind="ExternalInput").ap()
    OUT = nc.dram_tensor("out", [T, D], fp16, kind="ExternalOutput").ap()

    with tile.TileContext(nc) as tc, ExitStack() as ctx:
        const = ctx.enter_context(tc.tile_pool(name="const", bufs=1))
        big = ctx.enter_context(tc.tile_pool(name="big", bufs=1))
        et_pool = ctx.enter_context(tc.tile_pool(name="etp", bufs=6))
        r_pool = ctx.enter_context(tc.tile_pool(name="recip", bufs=2))
        fout_pool = ctx.enter_context(tc.tile_pool(name="fout", bufs=6))
        ps_pool = ctx.enter_context(tc.tile_pool(name="ps_pool", bufs=1,
                                                 space="PSUM"))

        # weights + x loads split finely across DMA queues (one queue moves
        # ~22.5 GB/s, so a monolithic 256KB+ DMA gates the pipeline start).
        wk_sb = const.tile([128, NDT, F], fp16)
        wq_sb = const.tile([128, NDT, F], fp16)
        wv_sb = const.tile([128, NDT, F], fp16)
        issuers = [nc.sync, nc.scalar]
        syncs = [nc.sync, nc.sync]
        xts = [big.tile([128, NDT, QB], fp16, name=f"xt{tb}")
               for tb in range(TPB)]
        for j in range(4):  # wk first (needed by the very first matmul)
            issuers[j % 2].dma_start(out=wk_sb[:, 2 * j:2 * j + 2, :],
                                     in_=WK[:, 2 * j:2 * j + 2, :])
        for j in range(NDT):  # xt0/xt1 8-way, issue spread over 2 engines
            issuers[j % 2].dma_start(out=xts[0][:, j:j + 1, :],
                                     in_=X[0][:, j:j + 1, :])
        for j in range(4):
            nc.scalar.dma_start(out=wq_sb[:, 2 * j:2 * j + 2, :],
                                in_=WQ[:, 2 * j:2 * j + 2, :])
            nc.sync.dma_start(out=wv_sb[:, 2 * j:2 * j + 2, :],
                              in_=WV[:, 2 * j:2 * j + 2, :])
        for j in range(NDT):
            issuers[j % 2].dma_start(out=xts[1][:, j:j + 1, :],
                                     in_=X[1][:, j:j + 1, :])
        bq_sb = const.tile([F, 1], f32)
        nc.scalar.dma_start(out=bq_sb, in_=BQ)
        for j in range(NDT):
            nc.sync.dma_start(out=xts[2][:, j:j + 1, :],
                              in_=X[2][:, j:j + 1, :])
        for tb in range(3, TPB):
            for j in range(NDT):
                nc.sync.dma_start(out=xts[tb][:, j:j + 1, :],
                                  in_=X[tb][:, j:j + 1, :])
        wfc_sb = const.tile([F, D], fp16)
        for j in range(4):
            nc.sync.dma_start(out=wfc_sb[:, j * 256:(j + 1) * 256],
                              in_=WFC[:, j * 256:(j + 1) * 256])

        qTs = [big.tile([128, QB], fp16, name=f"qT{i}") for i in range(TPB)]
        kTs = [big.tile([128, QB], fp16, name=f"kT{i}") for i in range(TPB)]
        # per token-block key-major V: [128 keys, 4 key-tiles, 192]
        # cols: [V_h0(0:64) | ones(64:128) | V_h1(128:192)]
        vkbs = [big.tile([128, 4, 192], fp16, name=f"vkb{i}")
                for i in range(TPB)]
        for vkb in vkbs:
            nc.gpsimd.memset(vkb[:, :, 64:128], 1.0)
        valuesTs = [big.tile([128, QB], fp16, name=f"valT{i}")
                    for i in range(TPB)]

        # ---------------- emission helpers ----------------
        def emit_qk_proj(tb, which):
            w_sb = wq_sb if which == "q" else wk_sb
            ps = ps_pool.tile([128, QB], f32, tag="mm", bufs=2,
                              name=f"{which}ps{tb}")
            for dt_ in range(NDT):
                nc.tensor.matmul(ps, w_sb[:, dt_, :], xts[tb][:, dt_, :],
                                 start=(dt_ == 0), stop=(dt_ == NDT - 1))
            if which == "q":
                nc.vector.tensor_scalar(qTs[tb], ps, bq_sb, 0.125,
                                        op0=OP.add, op1=OP.mult)
            else:
                nc.vector.tensor_copy(kTs[tb], ps)

        def emit_v_proj_half(tb, h, st):
            """V projection half (2 token-tiles); h=1 also evicts."""
            if h == 0:
                st["ps"] = ps_pool.tile([128, QB], f32, tag="mm", bufs=2,
                                        name=f"vps{tb}")
            ps = st["ps"]
            for sub in (2 * h, 2 * h + 1):
                for dt_ in range(NDT):
                    nc.tensor.matmul(
                        ps[:, sub * 128:(sub + 1) * 128],
                        xts[tb][:, dt_, sub * 128:(sub + 1) * 128],
                        wv_sb[:, dt_, :],
                        start=(dt_ == 0), stop=(dt_ == NDT - 1))
            if h == 1:
                vkb = vkbs[tb]
                psv = ps.rearrange("p (a c) -> p a c", c=128)
                nc.vector.tensor_copy(vkb[:, :, 0:64], psv[:, :, 0:64])
                nc.vector.tensor_copy(vkb[:, :, 128:192], psv[:, :, 64:128])

        def emit_v_proj(tb):
            st = {}
            emit_v_proj_half(tb, 0, st)
            emit_v_proj_half(tb, 1, st)

        def emit_norm(blk, pav, c0, c1):
            vt = valuesTs[blk]
            n = c1 - c0
            den0 = r_pool.tile([64, n], f32, tag="den0", name=f"den0_{blk}_{c0}")
            nc.vector.stream_shuffle(den0, pav[0][64:128, c0:c1], IDENT32)
            rec0 = r_pool.tile([64, n], f32, tag="rec0", name=f"rec0_{blk}_{c0}")
            nc.vector.reciprocal_approx_fast(out=rec0, in_=den0)
            nc.vector.tensor_mul(vt[0:64, c0:c1], pav[0][0:64, c0:c1], rec0)
            rec1 = r_pool.tile([64, n], f32, tag="rec1", name=f"rec1_{blk}_{c0}")
            nc.vector.reciprocal_approx_fast(out=rec1, in_=pav[1][0:64, c0:c1])
            rec1b = r_pool.tile([128, n], f32, tag="rec1b",
                                name=f"rec1b_{blk}_{c0}")
            nc.vector.stream_shuffle(rec1b[64:128, :], rec1, IDENT32)
            nc.vector.tensor_mul(vt[64:128, c0:c1], pav[1][64:128, c0:c1],
                                 rec1b[64:128, :])

        fc_in_tail = [False]
        tail_rr = [0]

        def emit_fc(b, tb2, eb):
            tt = b * S + tb2 * 128
            fp = ps_pool.tile([128, QB], f32, tag="mm", bufs=2,
                              name=f"fp{b}_{tb2}_{eb}")
            nc.tensor.matmul(
                fp,
                valuesTs[b * 4 + tb2 // 4][:, (tb2 % 4) * 128:(tb2 % 4 + 1) * 128],
                wfc_sb[:, eb * QB:(eb + 1) * QB],
                start=True, stop=True)
            fo = fout_pool.tile([128, QB], fp16, tag="fout",
                                name=f"fo{b}_{tb2}_{eb}")
            if fc_in_tail[0] and eb % 2 == 1:
                nc.scalar.copy(fo, fp)
            else:
                nc.vector.tensor_copy(fo, fp)
            # partition-half split keeps 1KB descriptors with 2x parallelism
            if fc_in_tail[0]:
                tail_rr[0] += 1
                iss = [nc.sync, nc.scalar, nc.gpsimd]
                iss[tail_rr[0] % 3].dma_start(
                    out=OUT[tt:tt + 64, eb * QB:(eb + 1) * QB], in_=fo[0:64, :])
                iss[(tail_rr[0] + 1) % 3].dma_start(
                    out=OUT[tt + 64:tt + 128, eb * QB:(eb + 1) * QB],
                    in_=fo[64:128, :])
            else:
                nc.sync.dma_start(out=OUT[tt:tt + 64, eb * QB:(eb + 1) * QB],
                                  in_=fo[0:64, :])
                nc.sync.dma_start(
                    out=OUT[tt + 64:tt + 128, eb * QB:(eb + 1) * QB],
                    in_=fo[64:128, :])

        # ---------------- static per-block work plan ----------------
        # plan[blk] = list of (cost_ns, fn); drained evenly over the block's
        # 16 kt steps.
        plan = [[] for _ in range(9)]  # plan[8] = tail

        class Pacer:
            def __init__(self):
                self.units = []
                self.done = 0.0
                self.total = 0.0
                self.kt = 0

            def block_start(self, blk):
                self.units = list(plan[blk])
                self.total = sum(u[0] for u in self.units) or 1.0
                self.done = 0.0
                self.kt = 0

            def step(self):
                self.kt += 1
                target = self.total * self.kt / (2 * NKT)
                while self.units and self.done < target:
                    c, fn = self.units.pop(0)
                    fn()
                    self.done += c

            def flush(self):
                for _, fn in self.units:
                    fn()
                self.units = []

        pacer = Pacer()

        def emit_attn_block(blk, prev_epilogue=None):
            b, qb = blk // 4, blk % 4
            qTq = qTs[b * 4 + qb]
            pav = [ps_pool.tile([128, QB], f32, tag=f"pav{h}",
                                name=f"pav{h}_{blk}") for h in range(HPC)]
            pending = []

            def emit_av(kt, et):
                vkb = vkbs[b * 4 + kt // 4]
                for h in range(HPC):
                    nc.tensor.matmul(
                        pav[h], vkb[:, kt % 4, h * 64:h * 64 + 128],
                        et[:, h * QB:(h + 1) * QB],
                        start=(kt == 0), stop=(kt == NKT - 1))

            for kt in range(NKT):
                # scores first: the exp chain is the pace-setter, so its
                # input must be produced as early as possible in the step.
                kTk = kTs[b * 4 + kt // 4]
                k0 = (kt % 4) * KT
                sc = ps_pool.tile([128, 2 * QB], f32, tag="sc", bufs=2,
                                  name=f"sc{blk}_{kt}")
                for h in range(HPC):
                    hp = h * HD
                    nc.tensor.matmul(
                        sc[:, h * QB:(h + 1) * QB],
                        kTk[hp:hp + HD, k0:k0 + KT],
                        qTq[hp:hp + HD, :],
                        start=True, stop=True,
                        tile_position=(hp, 0))
                et = et_pool.tile([128, 2 * QB], fp16, tag="et",
                                  name=f"et{blk}_{kt}")
                nc.scalar.activation(et, sc, AF.Exp)
                pending.append((kt, et))
                if kt == 0 and prev_epilogue is not None:
                    # previous block's last AV + norm ride here, after this
                    # block's first scores are already feeding the Act engine
                    prev_epilogue()
                if pending and kt >= AV_LAG:
                    emit_av(*pending.pop(0))
                if kt >= NKT - 3 and pending:
                    emit_av(*pending.pop(0))
                pacer.step()
                pacer.step()

            def epilogue():
                for item in pending:
                    emit_av(*item)
                emit_norm(blk, pav, 0, QB)
            if blk < 7:
                return epilogue

            for item in pending:
                emit_av(*item)
            # final block: normalize in quarters so the tail FC overlaps
            fc_in_tail[0] = True
            for q in range(4):
                emit_norm(blk, pav, q * 128, (q + 1) * 128)
                for e in range(D // QB):
                    emit_fc(1, 12 + q, e)

        # ---------------- master schedule ----------------
        C_PROJ, C_V, C_FC = 1900, 2600, 300

        def qk(tb, w):
            return (C_PROJ, lambda: emit_qk_proj(tb, w))

        def vp(tb):
            st = {}

            def half(h):
                def go():
                    emit_v_proj_half(tb, h, st)
                return go
            return [(C_V // 2, half(0)), (C_V // 2, half(1))]

        def fcs(blk):
            b, qb = blk // 4, blk % 4
            return [(C_FC, lambda b=b, t=t, e=e: emit_fc(b, t, e))
                    for t in range(qb * 4, qb * 4 + 4) for e in range(D // QB)]

        # pre: the minimum to start attention (first scores need K0+Q0,
        # first AVs need V0); everything else drains as paced units
        _pre = nc.named_scope("pre")
        _pre.__enter__()
        emit_qk_proj(0, "k")
        emit_qk_proj(0, "q")
        _pre.__exit__(None, None, None)

        plan[0] = (vp(0) + [qk(1, "k")] + vp(1) + [qk(2, "k")] + vp(2)
                   + [qk(3, "k")] + vp(3) + [qk(1, "q")])
        plan[1] = [qk(2, "q"), qk(4, "k")] + vp(4)
        plan[2] = [qk(3, "q"), qk(5, "k")] + vp(5) + [qk(6, "k")]
        plan[3] = [qk(4, "q")] + vp(6) + [qk(7, "k")] + vp(7)
        allfc = fcs(0) + fcs(1) + fcs(2) + fcs(3) + fcs(4) + fcs(5) + fcs(6)
        plan[4] = [qk(5, "q")] + allfc[0:16]
        plan[5] = [qk(6, "q")] + allfc[16:30]
        plan[6] = [qk(7, "q")] + allfc[30:44]
        plan[7] = allfc[44:56]
        plan[8] = []

        epi = None
        for blk in range(8):
            scope = nc.named_scope(f"attn{blk // 4}{blk % 4}")
            scope.__enter__()
            pacer.block_start(blk)
            epi = emit_attn_block(blk, prev_epilogue=epi)
            pacer.flush()
            scope.__exit__(None, None, None)
        _tail = nc.named_scope("tail")
        _tail.__enter__()
        fc_in_tail[0] = True
        for _, fn in plan[8]:
            fn()
        _tail.__exit__(None, None, None)

    nc.compile()
    return nc


def _get_nc():
    global _NC_CACHE
    if _NC_CACHE is None:
        _NC_CACHE = _build()
    return _NC_CACHE


def _prep_in_maps(x, w_qkv, b_qkv, w_fc):
    # x: [B,S,D] -> [TPB, 128, NDT, QB] : x[tb,p,dt,t] = xT[dt*128+p, tb*QB+t]
    xT = x.reshape(T, D).T.astype(np.float16)              # [D, T]
    xh = np.ascontiguousarray(
        xT.reshape(NDT, 128, TPB, QB).transpose(2, 1, 0, 3))

    def wlay(w):  # [D, F] -> [128, NDT, F]
        return np.ascontiguousarray(
            w.reshape(NDT, 128, F).transpose(1, 0, 2)).astype(np.float16)

    in_maps = []
    for c in range(NC):
        heads = [HPC * c + i for i in range(HPC)]
        rows = {
            "q": np.concatenate([np.arange(h * 3 * HD, h * 3 * HD + HD)
                                 for h in heads]),
            "k": np.concatenate([np.arange(h * 3 * HD + HD, h * 3 * HD + 2 * HD)
                                 for h in heads]),
            "v": np.concatenate([np.arange(h * 3 * HD + 2 * HD, h * 3 * HD + 3 * HD)
                                 for h in heads]),
        }
        m = {
            "x": xh,
            "wq": wlay(w_qkv[rows["q"]].T.astype(np.float16)),
            "wk": wlay(w_qkv[rows["k"]].T.astype(np.float16)),
            "wv": wlay(w_qkv[rows["v"]].T.astype(np.float16)),
            "bq": np.ascontiguousarray(b_qkv[rows["q"]][:, None]).astype(np.float32),
            "wfc": np.ascontiguousarray(
                w_fc[:, c * F:(c + 1) * F].T).astype(np.float16),
        }
        in_maps.append(m)
    return in_maps


def run_kernel(inputs, trace=False, trace_cores=None):
    x = np.asarray(inputs["x"], np.float32)
    w_qkv = np.asarray(inputs["w_qkv"], np.float32)
    b_qkv = np.asarray(inputs["b_qkv"], np.float32)
    w_fc = np.asarray(inputs["w_fc"], np.float32)
    b_fc = np.asarray(inputs["b_fc"], np.float32)

    nc = _get_nc()
    in_maps = _prep_in_maps(x, w_qkv, b_qkv, w_fc)
    res = run_bass_kernel_spmd(
        nc, in_maps, core_ids=list(range(NC)), trace=trace,
        trace_cores=trace_cores,
    )
    out = np.zeros((T, D), np.float32)
    for r in res.results:
        out += np.asarray(r["out"], np.float32)
    b_v_full = np.concatenate(
        [b_qkv[h * 3 * HD + 2 * HD:h * 3 * HD + 3 * HD] for h in range(H)])
    out += b_fc[None, :] + (w_fc @ b_v_full)[None, :]
    return out.reshape(B, S, D), res


def kernel(**inputs):
    out, _ = run_kernel(inputs, trace=False)
    return out
